# revision 8
# baseline (speedup 1.0000x reference)
"""Trainium2 Bass kernel for nn_DGNN (gnn_message_passing).

Reference computation (B=4, N=8192, F=32):
    delay_steps = time_delay // 5
    active      = (t >= delay_steps) & (adj > 0)
    A           = where(active, adj, 0)              # == adj * (time_delay <= 5*t+4)
    adjusted    = einsum('ij,bjf->bif', A, x)
    h           = relu(adjusted @ W1 + b1)
    out         = sigmoid(h @ W2 + b2)

Sharding / layout (host does layout-only transforms + dtype casts, no
reference math):
  - destination nodes i are split row-wise across 8 cores (1024 each);
  - adj/time_delay are shipped transposed ([j, i], j on partitions) because
    the PE contracts over the partition dim; both are additionally packed
    as [128, jt_n*ni] (partition p holds rows {p, p+128, ...}) so every
    chunk DMA is 128 fully-contiguous multi-KB partition lines;
  - adj is shipped fp16 (10-bit mantissa; measured end-to-end rel err
    ~1e-2 < 2e-2 budget) which halves the dominant HBM traffic and
    doubles PE throughput vs fp32; time_delay values 0..99 ship as int8
    (lossless narrowing; falls back to int32 otherwise);
  - x is repacked so the 4 batches sit side-by-side in the stationary
    operand (partition q = 32*b + f), giving full-width M=128 matmuls;
  - W1/W2 become 128x128 block-diagonal so the per-node MLP handles all 4
    batches in one matmul.

On-device per core: stream packed adj/td chunks (2 MB / 1 MB DMAs), one
fused DVE op per jt tile (TENSOR_MASK: out = select(td < thr+0.5, adj, 0))
produces the masked adjacency, fp16 matmuls accumulate adjusted^T over 64
K-tiles in fp32 PSUM, then the block-diagonal MLP and sigmoid run on-chip.
Output returns transposed per core and is unsharded on the host.
Per-core HBM traffic: 16 MB adj + 8 MB td + 2 MB x ~= 26.5 MB -> ~74 us
DMA roofline at 358 GB/s per core (this is the bottleneck; PE ~27 us).
"""

import numpy as np

B = 4
N = 8192
F = 32
P = 128
NCORES = 8
NI = N // NCORES  # dest-nodes per core
JT = N // P       # contraction tiles

MM_N = 512        # moving-operand free dim per matmul (PSUM bank limit)
CHUNK_JT = 8      # jt tiles per DMA chunk
ACT_MOD = 5       # jt % ACT_MOD < ACT_NUM goes via the scalar-engine route
ACT_NUM = 3       # (3/5 = 60% of tiles; balances ACT vs DVE busy time)
GATE_K = 4.0      # C = relu(GATE_K*(thr+0.5-td)); needs GATE_K/2 > max(adj)


def _round_fp32r(a):
    """Round fp32 to the fp32r grid (11 explicit mantissa bits, RNE)."""
    u = np.ascontiguousarray(a, dtype=np.float32).view(np.uint32)
    low = u & np.uint32(0xFFF)
    lsb = (u >> np.uint32(12)) & np.uint32(1)
    roundup = (low > 0x800) | ((low == 0x800) & (lsb == 1))
    u2 = (u & np.uint32(0xFFFFF000)) + (roundup.astype(np.uint32) << np.uint32(12))
    return u2.view(np.float32)


def _build(nj, ni, thr, mm_dtype_name="float16", td_dtype=np.int8,
           chunk_jt=CHUNK_JT, act_frac=(ACT_MOD, ACT_NUM)):
    """Trace + compile the per-core Bass program."""
    from contextlib import ExitStack

    import concourse.bacc as bacc
    import concourse.mybir as mybir
    import concourse.tile as tile
    from concourse.dve_ops import TENSOR_MASK

    f32 = mybir.dt.float32
    mm_dt = getattr(mybir.dt, mm_dtype_name)
    # adj travels in the matmul dtype's container (fp16 in fp16 mode,
    # fp32 otherwise -- float32r is an fp32 container truncated by the PE)
    ship_dt = mybir.dt.float16 if mm_dtype_name == "float16" else f32
    td_dt = mybir.dt.from_np(np.dtype(td_dtype))

    jt_n = nj // P
    mm_n = min(MM_N, ni)
    nh = ni // mm_n
    n_chunks = jt_n // chunk_jt
    cw = chunk_jt * ni  # chunk width in elements

    nc = bacc.Bacc("TRN2", target_bir_lowering=False, debug=False)

    adjP_d = nc.dram_tensor("adjP", [P, jt_n * ni], ship_dt, kind="ExternalInput").ap()
    tdP_d = nc.dram_tensor("tdP", [P, jt_n * ni], td_dt, kind="ExternalInput").ap()
    xsb_d = nc.dram_tensor("xsb", [P, jt_n * P], mm_dt, kind="ExternalInput").ap()
    bd1_d = nc.dram_tensor("bd1", [P, P], mm_dt, kind="ExternalInput").ap()
    bd2_d = nc.dram_tensor("bd2", [P, P], mm_dt, kind="ExternalInput").ap()
    bias1_d = nc.dram_tensor("bias1", [P, 1], f32, kind="ExternalInput").ap()
    bias2_d = nc.dram_tensor("bias2", [P, 1], f32, kind="ExternalInput").ap()
    outT_d = nc.dram_tensor("outT", [P, ni], f32, kind="ExternalOutput").ap()

    with tile.TileContext(nc) as tc, ExitStack() as ctx:
        io = ctx.enter_context(tc.tile_pool(name="io", bufs=4))
        wrk = ctx.enter_context(tc.tile_pool(name="wrk", bufs=6))
        singles = ctx.enter_context(tc.tile_pool(name="singles", bufs=1))
        pp = ctx.enter_context(tc.tile_pool(name="pp", bufs=1, space="PSUM"))

        x_t = singles.tile([P, jt_n * P], mm_dt)
        psum_main = pp.tile([P, ni], f32)
        bd1_t = singles.tile([P, P], mm_dt)
        bd2_t = singles.tile([P, P], mm_dt)
        bias1_t = singles.tile([P, 1], f32)
        bias2_t = singles.tile([P, 1], f32)
        gbias_t = singles.tile([P, 1], f32)
        gscale_t = singles.tile([P, 1], f32)
        warm_t = singles.tile([P, 1], f32)

        # gate constants + ACT Relu table warm before the stream starts
        nc.vector.memset(gbias_t, GATE_K * (float(thr) + 0.5))
        nc.vector.memset(gscale_t, -GATE_K)
        nc.vector.memset(warm_t, 0.0)
        nc.scalar.activation(
            warm_t, warm_t, mybir.ActivationFunctionType.Relu, bias=gbias_t,
            scale=gscale_t,
        )

        amod, anum = act_frac

        # PE executes in program order: start must clear PSUM on the first
        # ISSUED matmul and stop must be on the last issued one
        issued = [0]

        def do_mms(jt, a_t):
            lhsT = x_t[:, jt * P : (jt + 1) * P]
            for h in range(nh):
                nc.tensor.matmul(
                    psum_main[:, h * mm_n : (h + 1) * mm_n],
                    lhsT,
                    a_t[:, h * mm_n : (h + 1) * mm_n],
                    start=(issued[0] == 0),
                    stop=(issued[0] == jt_n - 1),
                )
            issued[0] += 1

        for ch in range(n_chunks):
            # adj rides the sync HWDGE ring alone; td/x ride the gpsimd
            # SWDGE ring; the scalar engine queue stays pure ACT compute so
            # gate ops never delay a DMA dispatch
            adj_t = io.tile([P, cw], ship_dt, tag="adj")
            nc.sync.dma_start(out=adj_t, in_=adjP_d[:, ch * cw : (ch + 1) * cw])
            td_t = io.tile([P, cw], td_dt, tag="td")
            nc.gpsimd.dma_start(out=td_t, in_=tdP_d[:, ch * cw : (ch + 1) * cw])
            # x slice used by this chunk's matmuls rides along
            xs = slice(ch * chunk_jt * P, (ch + 1) * chunk_jt * P)
            nc.gpsimd.dma_start(out=x_t[:, xs], in_=xsb_d[:, xs])

            if ch == 1:
                # small constants + sigmoid table pre-warm, off the critical path
                nc.sync.dma_start(out=bd1_t, in_=bd1_d)
                nc.sync.dma_start(out=bd2_t, in_=bd2_d)
                nc.sync.dma_start(out=bias1_t, in_=bias1_d)
                nc.sync.dma_start(out=bias2_t, in_=bias2_d)
                nc.scalar.activation(
                    warm_t, warm_t, mybir.ActivationFunctionType.Sigmoid,
                    bias=gbias_t,
                )

            # route split within the chunk: first dve_js (independent DVE
            # masks), then act_js whose TT-min waits on the ACT gate -- this
            # ordering avoids head-of-line blocking in the strict-FIFO DVE
            # queue while the ACT gates (issued first) run ahead
            act_js = [j for j in range(chunk_jt) if j % amod < anum]
            dve_js = [j for j in range(chunk_jt) if j % amod >= anum]

            c_ts = {}
            for j in act_js:
                js = slice(j * ni, (j + 1) * ni)
                # C = relu(K*(thr+0.5-td)) is a {0, >=2} gate
                c_t = wrk.tile([P, ni], mm_dt, tag="c")
                nc.scalar.activation(
                    c_t, td_t[:, js], mybir.ActivationFunctionType.Relu,
                    bias=gbias_t, scale=gscale_t,
                )
                c_ts[j] = c_t

            for j in dve_js:
                jt = ch * chunk_jt + j
                js = slice(j * ni, (j + 1) * ni)
                # TENSOR_MASK: out[k] = select(in1[k] + c2 < c0, in0[k], 0)
                a_t = wrk.tile([P, ni], mm_dt, tag="a")
                nc.vector._custom_dve(
                    TENSOR_MASK, out=a_t, in0=adj_t[:, js], in1=td_t[:, js],
                    s0=float(thr) + 0.5, s1=0.0, imm2=0.0,
                )
                do_mms(jt, a_t)

            for j in act_js:
                jt = ch * chunk_jt + j
                js = slice(j * ni, (j + 1) * ni)
                # A = min(adj, C) on DVE at the 2x 16-bit TT rate
                a_t = wrk.tile([P, ni], mm_dt, tag="a")
                nc.vector.tensor_tensor(
                    a_t, adj_t[:, js], c_ts[j], op=mybir.AluOpType.min
                )
                do_mms(jt, a_t)

        # Per-node MLP, pipelined in independent column halves.
        h_ps = pp.tile([P, ni], f32, tag="hps")
        o_ps = pp.tile([P, ni], f32, tag="ops")
        for h in range(nh):
            hs = slice(h * mm_n, (h + 1) * mm_n)
            res_t = singles.tile([P, mm_n], mm_dt, tag=f"res{h}", name=f"res{h}")
            nc.vector.tensor_copy(res_t, psum_main[:, hs])
            nc.tensor.matmul(h_ps[:, hs], bd1_t, res_t, start=True, stop=True)
            # h = relu(. + b1) fused on DVE: (in + bias) max 0
            h_t = singles.tile([P, mm_n], mm_dt, tag=f"h{h}", name=f"h{h}")
            nc.vector.tensor_scalar(
                h_t, h_ps[:, hs], bias1_t, 0.0,
                op0=mybir.AluOpType.add,
                op1=mybir.AluOpType.max,
            )
            nc.tensor.matmul(o_ps[:, hs], bd2_t, h_t, start=True, stop=True)
            out_t = singles.tile([P, mm_n], f32, tag=f"out{h}", name=f"out{h}")
            nc.scalar.activation(
                out_t, o_ps[:, hs], mybir.ActivationFunctionType.Sigmoid, bias=bias2_t
            )
            nc.sync.dma_start(out=outT_d[:, hs], in_=out_t)

    nc.compile()
    return nc


def _host_prep(x, adj, time_delay, t, W1, b1, W2, b2, ncores, rnd, td_dtype,
               ship_np):
    """Layout-only transforms (transpose / repack / dtype container casts)."""
    x = np.ascontiguousarray(np.asarray(x, dtype=np.float32))
    adj = np.asarray(adj, dtype=np.float32)
    td = np.asarray(time_delay)
    b, n, f = x.shape
    ni = n // ncores
    jt_n = n // P

    thr = int(t) * 5 + 4  # time_delay // 5 <= t  <=>  time_delay <= 5t+4

    # packed transposed layouts: arr[p, jt*ni + c] = src.T[jt*128 + p, c]
    adjT = adj.T.astype(ship_np)                      # [n, n] (j, i)
    tdT = td.T.astype(td_dtype)
    # stationary x: x_sb[p, jt*P + 32*b + f] = x[b, jt*P + p, f]
    xsb = rnd(
        x.reshape(b, jt_n, P, f).transpose(2, 1, 0, 3).reshape(P, jt_n * b * f)
    )
    bd1 = np.zeros((P, P), np.float32)
    bd2 = np.zeros((P, P), np.float32)
    for bb in range(b):
        bd1[bb * f : (bb + 1) * f, bb * f : (bb + 1) * f] = W1
        bd2[bb * f : (bb + 1) * f, bb * f : (bb + 1) * f] = W2
    bd1 = rnd(bd1)
    bd2 = rnd(bd2)
    bias1 = np.ascontiguousarray(np.tile(np.asarray(b1, np.float32), b).reshape(P, 1))
    bias2 = np.ascontiguousarray(np.tile(np.asarray(b2, np.float32), b).reshape(P, 1))

    in_maps = []
    for c in range(ncores):
        sl = slice(c * ni, (c + 1) * ni)
        # pack [jt_n*128, ni] -> [128, jt_n*ni] so each chunk DMA reads one
        # contiguous multi-KB span per partition
        adjP = np.ascontiguousarray(
            adjT[:, sl].reshape(jt_n, P, ni).transpose(1, 0, 2).reshape(P, jt_n * ni)
        )
        tdP = np.ascontiguousarray(
            tdT[:, sl].reshape(jt_n, P, ni).transpose(1, 0, 2).reshape(P, jt_n * ni)
        )
        in_maps.append(
            {
                "adjP": adjP,
                "tdP": tdP,
                "xsb": xsb,
                "bd1": bd1,
                "bd2": bd2,
                "bias1": bias1,
                "bias2": bias2,
            }
        )
    return thr, in_maps


def _run(x, adj, time_delay, t, W1, b1, W2, b2, ncores=NCORES,
         mm_dtype_name="float16", trace=False, chunk_jt=CHUNK_JT):
    from concourse.bass_utils import run_bass_kernel_spmd

    b, n, f = np.asarray(x).shape
    ni = n // ncores
    td = np.asarray(time_delay)
    # int8 shipping is only a container change; keep int32 when values
    # (or the threshold compare range) would not fit exactly.
    thr_chk = int(t) * 5 + 4
    if td.min() >= -127 and td.max() <= 127 and -127 <= thr_chk <= 127:
        td_dtype = np.int8
    else:
        td_dtype = np.int32
    if mm_dtype_name == "float16":
        rnd = lambda a: np.ascontiguousarray(a, dtype=np.float16)  # noqa: E731
        ship_np = np.float16
    elif mm_dtype_name == "float32r":
        rnd = _round_fp32r
        ship_np = np.float32
    else:
        rnd = lambda a: np.ascontiguousarray(a, dtype=np.float32)  # noqa: E731
        ship_np = np.float32
    thr, in_maps = _host_prep(
        x, adj, time_delay, t, W1, b1, W2, b2, ncores, rnd, td_dtype, ship_np
    )
    # the min-gate route assumes 0 <= adj < GATE_K/2; fall back to the
    # all-DVE TENSOR_MASK route otherwise (correct for any adj)
    adj_np = np.asarray(adj)
    if (mm_dtype_name == "float16" and float(adj_np.min()) >= 0.0
            and float(adj_np.max()) < GATE_K / 2):
        act_frac = (ACT_MOD, ACT_NUM)
    else:
        act_frac = (1, 0)
    nc = _build(n, ni, thr, mm_dtype_name, td_dtype, chunk_jt, act_frac)
    res = run_bass_kernel_spmd(
        nc, in_maps, core_ids=list(range(ncores)), trace=trace
    )
    full = np.concatenate([r["outT"] for r in res.results], axis=1)  # [P, n]
    out = np.ascontiguousarray(full.reshape(b, f, n).transpose(0, 2, 1))
    return out, res


def kernel(x, adj, time_delay, t, W1, b1, W2, b2):
    out, _ = _run(x, adj, time_delay, t, W1, b1, W2, b2)
    return out


# revision 9
# speedup vs baseline: 1.1939x; 1.1939x over previous
"""Trainium2 Bass kernel for nn_DGNN (gnn_message_passing).

Reference computation (B=4, N=8192, F=32):
    delay_steps = time_delay // 5
    active      = (t >= delay_steps) & (adj > 0)
    A           = where(active, adj, 0)              # == adj * (time_delay <= 5*t+4)
    adjusted    = einsum('ij,bjf->bif', A, x)
    h           = relu(adjusted @ W1 + b1)
    out         = sigmoid(h @ W2 + b2)

Sharding / layout (host does layout-only transforms + dtype casts, no
reference math):
  - destination nodes i are split row-wise across 8 cores (1024 each);
  - adj/time_delay are shipped transposed ([j, i], j on partitions) because
    the PE contracts over the partition dim; both are additionally packed
    as [128, jt_n*ni] (partition p holds rows {p, p+128, ...}) so every
    chunk DMA is 128 fully-contiguous multi-KB partition lines;
  - adj is shipped fp16 (10-bit mantissa; measured end-to-end rel err
    ~1e-2 < 2e-2 budget) which halves the dominant HBM traffic and
    doubles PE throughput vs fp32; time_delay values 0..99 ship as int8
    (lossless narrowing; falls back to int32 otherwise);
  - x is repacked so the 4 batches sit side-by-side in the stationary
    operand (partition q = 32*b + f), giving full-width M=128 matmuls;
  - W1/W2 become 128x128 block-diagonal so the per-node MLP handles all 4
    batches in one matmul.

On-device per core: stream packed adj/td chunks (2 MB / 1 MB DMAs), one
fused DVE op per jt tile (TENSOR_MASK: out = select(td < thr+0.5, adj, 0))
produces the masked adjacency, fp16 matmuls accumulate adjusted^T over 64
K-tiles in fp32 PSUM, then the block-diagonal MLP and sigmoid run on-chip.
Output returns transposed per core and is unsharded on the host.
Per-core HBM traffic: 16 MB adj + 8 MB td + 2 MB x ~= 26.5 MB -> ~74 us
DMA roofline at 358 GB/s per core (this is the bottleneck; PE ~27 us).
"""

import numpy as np

B = 4
N = 8192
F = 32
P = 128
NCORES = 8
NI = N // NCORES  # dest-nodes per core
JT = N // P       # contraction tiles

MM_N = 512        # moving-operand free dim per matmul (PSUM bank limit)
CHUNK_JT = 8      # jt tiles per DMA chunk
ACT_MOD = 5       # jt % ACT_MOD < ACT_NUM goes via the scalar-engine route
ACT_NUM = 3       # (3/5 = 60% of tiles; balances ACT vs DVE busy time)
GATE_K = 4.0      # C = relu(GATE_K*(thr+0.5-td)); needs GATE_K/2 > max(adj)


def _round_fp32r(a):
    """Round fp32 to the fp32r grid (11 explicit mantissa bits, RNE)."""
    u = np.ascontiguousarray(a, dtype=np.float32).view(np.uint32)
    low = u & np.uint32(0xFFF)
    lsb = (u >> np.uint32(12)) & np.uint32(1)
    roundup = (low > 0x800) | ((low == 0x800) & (lsb == 1))
    u2 = (u & np.uint32(0xFFFFF000)) + (roundup.astype(np.uint32) << np.uint32(12))
    return u2.view(np.float32)


def _build(nj, ni, thr, mm_dtype_name="float16", td_dtype=np.int8,
           chunk_jt=CHUNK_JT, act_frac=(ACT_MOD, ACT_NUM)):
    """Trace + compile the per-core Bass program."""
    from contextlib import ExitStack

    import concourse.bacc as bacc
    import concourse.mybir as mybir
    import concourse.tile as tile
    from concourse.dve_ops import TENSOR_MASK

    f32 = mybir.dt.float32
    mm_dt = getattr(mybir.dt, mm_dtype_name)
    # adj travels in the matmul dtype's container (fp16 in fp16 mode,
    # fp32 otherwise -- float32r is an fp32 container truncated by the PE)
    ship_dt = mybir.dt.float16 if mm_dtype_name == "float16" else f32
    td_dt = mybir.dt.from_np(np.dtype(td_dtype))

    jt_n = nj // P
    mm_n = min(MM_N, ni)
    nh = ni // mm_n
    n_chunks = jt_n // chunk_jt
    cw = chunk_jt * ni  # chunk width in elements

    nc = bacc.Bacc("TRN2", target_bir_lowering=False, debug=False)

    adjP_d = nc.dram_tensor("adjP", [P, jt_n * ni], ship_dt, kind="ExternalInput").ap()
    tdP_d = nc.dram_tensor("tdP", [P, jt_n * ni], td_dt, kind="ExternalInput").ap()
    xsb_d = nc.dram_tensor("xsb", [P, jt_n * P], mm_dt, kind="ExternalInput").ap()
    bd1_d = nc.dram_tensor("bd1", [P, P], mm_dt, kind="ExternalInput").ap()
    bd2_d = nc.dram_tensor("bd2", [P, P], mm_dt, kind="ExternalInput").ap()
    bias1_d = nc.dram_tensor("bias1", [P, 1], f32, kind="ExternalInput").ap()
    bias2_d = nc.dram_tensor("bias2", [P, 1], f32, kind="ExternalInput").ap()
    outT_d = nc.dram_tensor("outT", [P, ni], f32, kind="ExternalOutput").ap()

    with tile.TileContext(nc) as tc, ExitStack() as ctx:
        io = ctx.enter_context(tc.tile_pool(name="io", bufs=4))
        wrk = ctx.enter_context(tc.tile_pool(name="wrk", bufs=6))
        singles = ctx.enter_context(tc.tile_pool(name="singles", bufs=1))
        pp = ctx.enter_context(tc.tile_pool(name="pp", bufs=1, space="PSUM"))

        x_t = singles.tile([P, jt_n * P], mm_dt)
        psum_main = pp.tile([P, ni], f32)
        bd1_t = singles.tile([P, P], mm_dt)
        bd2_t = singles.tile([P, P], mm_dt)
        bias1_t = singles.tile([P, 1], f32)
        bias2_t = singles.tile([P, 1], f32)
        gbias_t = singles.tile([P, 1], f32)
        gscale_t = singles.tile([P, 1], f32)
        warm_t = singles.tile([P, 1], f32)

        # gate constants + ACT Relu table warm before the stream starts
        nc.vector.memset(gbias_t, GATE_K * (float(thr) + 0.5))
        nc.vector.memset(gscale_t, -GATE_K)
        nc.vector.memset(warm_t, 0.0)
        nc.scalar.activation(
            warm_t, warm_t, mybir.ActivationFunctionType.Relu, bias=gbias_t,
            scale=gscale_t,
        )

        amod, anum = act_frac

        # PE executes in program order: start must clear PSUM on the first
        # ISSUED matmul and stop must be on the last issued one
        issued = [0]

        def do_mms(jt, a_t):
            lhsT = x_t[:, jt * P : (jt + 1) * P]
            for h in range(nh):
                nc.tensor.matmul(
                    psum_main[:, h * mm_n : (h + 1) * mm_n],
                    lhsT,
                    a_t[:, h * mm_n : (h + 1) * mm_n],
                    start=(issued[0] == 0),
                    stop=(issued[0] == jt_n - 1),
                )
            issued[0] += 1

        tiles = {}

        def issue_dmas(c):
            # alternate the two HWDGE rings; both stream continuously
            qa, qb = (nc.scalar, nc.sync) if c % 2 == 0 else (nc.sync, nc.scalar)
            adj_t = io.tile([P, cw], ship_dt, tag="adj")
            qa.dma_start(out=adj_t, in_=adjP_d[:, c * cw : (c + 1) * cw])
            td_t = io.tile([P, cw], td_dt, tag="td")
            qb.dma_start(out=td_t, in_=tdP_d[:, c * cw : (c + 1) * cw])
            xs = slice(c * chunk_jt * P, (c + 1) * chunk_jt * P)
            qb.dma_start(out=x_t[:, xs], in_=xsb_d[:, xs])
            tiles[c] = (adj_t, td_t)

        for ch in range(n_chunks):
            # dispatch DMAs two chunks ahead of compute so ACT gate ops in
            # the scalar queue never delay a pending DMA dispatch
            if ch == 0:
                issue_dmas(0)
                issue_dmas(1)
            if ch + 2 < n_chunks:
                issue_dmas(ch + 2)
            adj_t, td_t = tiles.pop(ch)

            if ch == 1:
                # small constants + sigmoid table pre-warm, off the critical path
                nc.sync.dma_start(out=bd1_t, in_=bd1_d)
                nc.sync.dma_start(out=bd2_t, in_=bd2_d)
                nc.sync.dma_start(out=bias1_t, in_=bias1_d)
                nc.sync.dma_start(out=bias2_t, in_=bias2_d)
                nc.scalar.activation(
                    warm_t, warm_t, mybir.ActivationFunctionType.Sigmoid,
                    bias=gbias_t,
                )

            # route split within the chunk: first dve_js (independent DVE
            # masks), then act_js whose TT-min waits on the ACT gate -- this
            # ordering avoids head-of-line blocking in the strict-FIFO DVE
            # queue while the ACT gates (issued first) run ahead
            act_js = [j for j in range(chunk_jt) if j % amod < anum]
            dve_js = [j for j in range(chunk_jt) if j % amod >= anum]

            c_ts = {}
            for j in act_js:
                js = slice(j * ni, (j + 1) * ni)
                # C = relu(K*(thr+0.5-td)) is a {0, >=2} gate
                c_t = wrk.tile([P, ni], mm_dt, tag="c")
                nc.scalar.activation(
                    c_t, td_t[:, js], mybir.ActivationFunctionType.Relu,
                    bias=gbias_t, scale=gscale_t,
                )
                c_ts[j] = c_t

            for j in dve_js:
                jt = ch * chunk_jt + j
                js = slice(j * ni, (j + 1) * ni)
                # TENSOR_MASK: out[k] = select(in1[k] + c2 < c0, in0[k], 0)
                a_t = wrk.tile([P, ni], mm_dt, tag="a")
                nc.vector._custom_dve(
                    TENSOR_MASK, out=a_t, in0=adj_t[:, js], in1=td_t[:, js],
                    s0=float(thr) + 0.5, s1=0.0, imm2=0.0,
                )
                do_mms(jt, a_t)

            for j in act_js:
                jt = ch * chunk_jt + j
                js = slice(j * ni, (j + 1) * ni)
                # A = min(adj, C) on DVE at the 2x 16-bit TT rate
                a_t = wrk.tile([P, ni], mm_dt, tag="a")
                nc.vector.tensor_tensor(
                    a_t, adj_t[:, js], c_ts[j], op=mybir.AluOpType.min
                )
                do_mms(jt, a_t)

        # Per-node MLP, pipelined in independent column halves.
        h_ps = pp.tile([P, ni], f32, tag="hps")
        o_ps = pp.tile([P, ni], f32, tag="ops")
        for h in range(nh):
            hs = slice(h * mm_n, (h + 1) * mm_n)
            res_t = singles.tile([P, mm_n], mm_dt, tag=f"res{h}", name=f"res{h}")
            nc.vector.tensor_copy(res_t, psum_main[:, hs])
            nc.tensor.matmul(h_ps[:, hs], bd1_t, res_t, start=True, stop=True)
            # h = relu(. + b1) fused on DVE: (in + bias) max 0
            h_t = singles.tile([P, mm_n], mm_dt, tag=f"h{h}", name=f"h{h}")
            nc.vector.tensor_scalar(
                h_t, h_ps[:, hs], bias1_t, 0.0,
                op0=mybir.AluOpType.add,
                op1=mybir.AluOpType.max,
            )
            nc.tensor.matmul(o_ps[:, hs], bd2_t, h_t, start=True, stop=True)
            out_t = singles.tile([P, mm_n], f32, tag=f"out{h}", name=f"out{h}")
            nc.scalar.activation(
                out_t, o_ps[:, hs], mybir.ActivationFunctionType.Sigmoid, bias=bias2_t
            )
            nc.sync.dma_start(out=outT_d[:, hs], in_=out_t)

    nc.compile()
    return nc


def _host_prep(x, adj, time_delay, t, W1, b1, W2, b2, ncores, rnd, td_dtype,
               ship_np):
    """Layout-only transforms (transpose / repack / dtype container casts)."""
    x = np.ascontiguousarray(np.asarray(x, dtype=np.float32))
    adj = np.asarray(adj, dtype=np.float32)
    td = np.asarray(time_delay)
    b, n, f = x.shape
    ni = n // ncores
    jt_n = n // P

    thr = int(t) * 5 + 4  # time_delay // 5 <= t  <=>  time_delay <= 5t+4

    # packed transposed layouts: arr[p, jt*ni + c] = src.T[jt*128 + p, c]
    adjT = adj.T.astype(ship_np)                      # [n, n] (j, i)
    tdT = td.T.astype(td_dtype)
    # stationary x: x_sb[p, jt*P + 32*b + f] = x[b, jt*P + p, f]
    xsb = rnd(
        x.reshape(b, jt_n, P, f).transpose(2, 1, 0, 3).reshape(P, jt_n * b * f)
    )
    bd1 = np.zeros((P, P), np.float32)
    bd2 = np.zeros((P, P), np.float32)
    for bb in range(b):
        bd1[bb * f : (bb + 1) * f, bb * f : (bb + 1) * f] = W1
        bd2[bb * f : (bb + 1) * f, bb * f : (bb + 1) * f] = W2
    bd1 = rnd(bd1)
    bd2 = rnd(bd2)
    bias1 = np.ascontiguousarray(np.tile(np.asarray(b1, np.float32), b).reshape(P, 1))
    bias2 = np.ascontiguousarray(np.tile(np.asarray(b2, np.float32), b).reshape(P, 1))

    in_maps = []
    for c in range(ncores):
        sl = slice(c * ni, (c + 1) * ni)
        # pack [jt_n*128, ni] -> [128, jt_n*ni] so each chunk DMA reads one
        # contiguous multi-KB span per partition
        adjP = np.ascontiguousarray(
            adjT[:, sl].reshape(jt_n, P, ni).transpose(1, 0, 2).reshape(P, jt_n * ni)
        )
        tdP = np.ascontiguousarray(
            tdT[:, sl].reshape(jt_n, P, ni).transpose(1, 0, 2).reshape(P, jt_n * ni)
        )
        in_maps.append(
            {
                "adjP": adjP,
                "tdP": tdP,
                "xsb": xsb,
                "bd1": bd1,
                "bd2": bd2,
                "bias1": bias1,
                "bias2": bias2,
            }
        )
    return thr, in_maps


def _run(x, adj, time_delay, t, W1, b1, W2, b2, ncores=NCORES,
         mm_dtype_name="float16", trace=False, chunk_jt=CHUNK_JT):
    from concourse.bass_utils import run_bass_kernel_spmd

    b, n, f = np.asarray(x).shape
    ni = n // ncores
    td = np.asarray(time_delay)
    # int8 shipping is only a container change; keep int32 when values
    # (or the threshold compare range) would not fit exactly.
    thr_chk = int(t) * 5 + 4
    if td.min() >= -127 and td.max() <= 127 and -127 <= thr_chk <= 127:
        td_dtype = np.int8
    else:
        td_dtype = np.int32
    if mm_dtype_name == "float16":
        rnd = lambda a: np.ascontiguousarray(a, dtype=np.float16)  # noqa: E731
        ship_np = np.float16
    elif mm_dtype_name == "float32r":
        rnd = _round_fp32r
        ship_np = np.float32
    else:
        rnd = lambda a: np.ascontiguousarray(a, dtype=np.float32)  # noqa: E731
        ship_np = np.float32
    thr, in_maps = _host_prep(
        x, adj, time_delay, t, W1, b1, W2, b2, ncores, rnd, td_dtype, ship_np
    )
    # the min-gate route assumes 0 <= adj < GATE_K/2; fall back to the
    # all-DVE TENSOR_MASK route otherwise (correct for any adj)
    adj_np = np.asarray(adj)
    if (mm_dtype_name == "float16" and float(adj_np.min()) >= 0.0
            and float(adj_np.max()) < GATE_K / 2):
        act_frac = (ACT_MOD, ACT_NUM)
    else:
        act_frac = (1, 0)
    nc = _build(n, ni, thr, mm_dtype_name, td_dtype, chunk_jt, act_frac)
    res = run_bass_kernel_spmd(
        nc, in_maps, core_ids=list(range(ncores)), trace=trace
    )
    full = np.concatenate([r["outT"] for r in res.results], axis=1)  # [P, n]
    out = np.ascontiguousarray(full.reshape(b, f, n).transpose(0, 2, 1))
    return out, res


def kernel(x, adj, time_delay, t, W1, b1, W2, b2):
    out, _ = _run(x, adj, time_delay, t, W1, b1, W2, b2)
    return out
